# revision 48
# baseline (speedup 1.0000x reference)
"""AttentionSequencePoolingLayer on 8 TRN2 NeuronCores (Bass/Tile), v2.

Math (per batch b):
  att_in = [q, k, q-k, q*k] @ W1 + b1 -> sigmoid -> @W2+b2 -> sigmoid -> @W3+b3
  scores masked -> softmax over T -> attn @ keys

Folding: att_in @ W1 = k @ Weff_b + c_b, with
  Weff_b = (W1k - W1m) + q_b * W1p   (per-batch effective weight, [64,8])
  c_b    = q_b @ (W1q + W1m) + b1    (per-batch bias, [8])
Sigmoids are computed as tanh: sigma(x) = 0.5 + 0.5*tanh(x/2), with the
affine part folded into host-prepared W2/W3/biases so every activation
(tanh, exp) lives in one HW act-func set (exp_and_others) -> a single
LoadActFuncSet for the whole kernel.

Per core (512 batches, data-parallel over batch), per 64-batch tile:
  - ksup: t-major keys bf16 [100, 8192] (one DMA, 16KB/partition lines):
    col 1024*s8 + 128*bb + 64*r + d holds keys[8*s8+bb, 2p+r, d]. Feeds
    pooling only.
  - kt4: feature-major keys fp8(e4m3) [128, 6400] (one DMA): per quad q4
    cols 400*q4 + 200*i + tp, partition 64*h + d = keys[4*q4+2*i+h, t, d]
    with tp the parity-permuted t (tp<100 <-> t=2*tp, else t=2*(tp-100)+1).
    Feeds mm1 directly -- no PE transposes, no psum->sbuf key copies.
  - mm1: fp8 DoubleRow, one matmul per quad (contraction 256 = 4 batches
    x 64 features over 2 k-slots): lhsT weff [128,2,32], rhs [128,2,200],
    out [32,200] -> ps1 [128,200] = 16 batches.
  - t1 = tanh(0.5*ps1 + 0.5*c) -> bf16; mm2 block-diag [128,128] -> ps2;
    t2 = tanh(0.5*ps2 + b2fold); mm3 -> ps3 rows 32g..32g+16 (scoresT,
    parity-permuted t cols; masks host-permuted to match).
  - softmax along free dim; exp row-sums via ACT accum_out; attn -> bf16.
  - attn transposed (2 PE transposes, bf16) -> at [100, 256].
  - pooling: per 8-batch super 2 matmuls free=512 (attn columns
    stationary, strided sup views moving), even/odd accumulated in psum;
    batch j of a strip lands at psum[32q+j, 64j:64j+64] (diagonal).
  - psum -> sbuf copy on the (otherwise idle) Pool engine; one DMA per
    bank stores the 32 valid rows x 512 cols; the host extracts the
    diagonal 64-blocks for free.
"""
import os
import sys
import numpy as np

for _p in ("/opt/trn_rl_repo",):
    if os.path.isdir(_p) and _p not in sys.path:
        sys.path.insert(0, _p)

import ml_dtypes  # noqa: E402
from contextlib import ExitStack  # noqa: E402
import concourse.bass as bass  # noqa: E402
import concourse.tile as tile  # noqa: E402
from concourse import bacc, mybir  # noqa: E402
from concourse.bass_utils import run_bass_kernel_spmd  # noqa: E402

B, T, D = 4096, 200, 64
NCORES = 8
BC = B // NCORES            # 512 batches per core
NEG = np.float32(-2**32 + 1)
BF16 = mybir.dt.bfloat16
F32 = mybir.dt.float32
FP8 = mybir.dt.float8e4
U8 = mybir.dt.uint8
NPFP8 = ml_dtypes.float8_e4m3
TT_BATCHES = 64             # batches per tile
NTT = BC // TT_BATCHES      # 8


def _build_kernel(ntt=NTT):
    nbatch = ntt * TT_BATCHES
    nc = bacc.Bacc("TRN2", target_bir_lowering=False, debug=False,
                   num_devices=NCORES)
    ksup_d = nc.dram_tensor("ksup", [ntt, 100, 8192], BF16,
                            kind="ExternalInput").ap()
    kt4_d = nc.dram_tensor("kt4", [ntt, 128, 6400], FP8,
                           kind="ExternalInput").ap()
    weff_d = nc.dram_tensor("weff", [128, nbatch * 16], FP8,
                            kind="ExternalInput").ap()
    cbias_d = nc.dram_tensor("cbias", [128, ntt * 4], F32,
                             kind="ExternalInput").ap()
    bd2_d = nc.dram_tensor("bd2", [128, 128], BF16, kind="ExternalInput").ap()
    bd3_d = nc.dram_tensor("bd3", [128, 32], BF16, kind="ExternalInput").ap()
    b2v_d = nc.dram_tensor("b2v", [128, 1], F32, kind="ExternalInput").ap()
    masks_d = nc.dram_tensor("masks", [128, ntt * T], U8,
                             kind="ExternalInput").ap()
    id128_d = nc.dram_tensor("id128", [128, 128], BF16,
                             kind="ExternalInput").ap()
    out_d = nc.dram_tensor("out", [ntt, 128, 1024], BF16,
                           kind="ExternalOutput").ap()
    zsum_d = nc.dram_tensor("zsum", [128, ntt], F32,
                            kind="ExternalOutput").ap()

    with tile.TileContext(nc) as tc:
        with ExitStack() as ctx:
            _body(ctx, tc, ntt, ksup_d, kt4_d, weff_d, cbias_d, bd2_d, bd3_d,
                  b2v_d, masks_d, id128_d, out_d, zsum_d)
    nc.compile()
    return nc


def _body(ctx, tc, ntt, ksup_d, kt4_d, weff_d, cbias_d, bd2_d, bd3_d,
          b2v_d, masks_d, id128_d, out_d, zsum_d):
    nc = tc.nc
    Tanh = mybir.ActivationFunctionType.Tanh
    Exp = mybir.ActivationFunctionType.Exp
    DR = mybir.MatmulPerfMode.DoubleRow
    X = mybir.AxisListType.X
    MAX = mybir.AluOpType.max

    const = ctx.enter_context(tc.tile_pool(name="const", bufs=1))
    sup_pool = ctx.enter_context(tc.tile_pool(name="sup", bufs=8))
    kt4_pool = ctx.enter_context(tc.tile_pool(name="kt4p", bufs=8))
    t1_pool = ctx.enter_context(tc.tile_pool(name="t1", bufs=2))
    t2_pool = ctx.enter_context(tc.tile_pool(name="t2", bufs=2))
    sm_pool = ctx.enter_context(tc.tile_pool(name="sm", bufs=2))
    at_pool = ctx.enter_context(tc.tile_pool(name="at", bufs=9))
    stg_pool = ctx.enter_context(tc.tile_pool(name="stg", bufs=3))
    ps_1 = ctx.enter_context(tc.tile_pool(name="ps1", bufs=2, space="PSUM"))
    ps_2 = ctx.enter_context(tc.tile_pool(name="ps2", bufs=2, space="PSUM"))
    ps_3 = ctx.enter_context(tc.tile_pool(name="ps3", bufs=1, space="PSUM"))
    ps_a = ctx.enter_context(tc.tile_pool(name="psa", bufs=1, space="PSUM"))
    ps_p = ctx.enter_context(tc.tile_pool(name="psp", bufs=2, space="PSUM"))

    # constants (scalar-engine HWDGE; sync engine streams the big key DMAs)
    weff = const.tile([128, weff_d.shape[1]], FP8)
    wsl = weff_d.shape[1] // ntt
    for tt in range(ntt):
        nc.scalar.dma_start(weff[:, wsl * tt:wsl * (tt + 1)],
                            weff_d[:, wsl * tt:wsl * (tt + 1)])
    cbias = const.tile([128, cbias_d.shape[1]], F32)
    nc.scalar.dma_start(cbias[:], cbias_d[:])
    bd2 = const.tile([128, 128], BF16)
    nc.scalar.dma_start(bd2[:], bd2_d[:])
    bd3 = const.tile([128, 32], BF16)
    nc.scalar.dma_start(bd3[:], bd3_d[:])
    b2v = const.tile([128, 1], F32)
    nc.scalar.dma_start(b2v[:], b2v_d[:])
    mask_t = const.tile([128, ntt * T], U8)
    nc.scalar.dma_start(mask_t[:], masks_d[:])
    id128 = const.tile([128, 128], BF16)
    nc.scalar.dma_start(id128[:], id128_d[:])
    negC = const.tile([128, T], F32)
    nc.vector.memset(negC[:], -30.0)
    zstg = const.tile([128, ntt], F32)

    # Two phases. Phase 1: stream the (small) kt4 tiles and run every
    # tile's MLP+softmax+attn-transpose -- latency-bound, finishes while
    # the sup stream is still arriving. Phase 2: stream the (big) sup
    # tiles and run pooling paced by their arrival. The DMA queue stays
    # saturated: all kt4 DMAs are issued first, the sup DMAs queue right
    # behind them.
    ats = {}
    prev = None
    for tt in range(ntt):
        kt4 = kt4_pool.tile([128, 6400], FP8, tag="kt4")
        nc.sync.dma_start(kt4[:], kt4_d[tt])
        cur = _mlp_softmax(nc, tc, ntt, tt, kt4, weff, cbias, bd2,
                           bd3, b2v, mask_t, negC, zstg, ps_1, ps_2,
                           ps_3, t1_pool, t2_pool, sm_pool)
        # transpose of the PREVIOUS tile's exp-weights: its exp finished
        # while this tile's MLP matmuls ran, so the PE never stalls.
        if prev is not None:
            ats[prev[0]] = _attn_transp(nc, prev[1], id128, ps_a, at_pool)
        prev = cur
    ats[prev[0]] = _attn_transp(nc, prev[1], id128, ps_a, at_pool)

    for tt in range(ntt):
        sup = sup_pool.tile([100, 8192], BF16, tag="sup")
        nc.sync.dma_start(sup[:], ksup_d[tt])
        _pool_out(nc, tt, sup, ats[tt], ps_p, stg_pool, out_d)
    nc.gpsimd.dma_start(zsum_d[:], zstg[:])


def _mlp_softmax(nc, tc, ntt, tt, kt4, weff, cbias, bd2, bd3, b2v,
                 mask_t, negC, zstg, ps_1, ps_2, ps_3, t1_pool, t2_pool,
                 sm_pool):
    Tanh = mybir.ActivationFunctionType.Tanh
    Exp = mybir.ActivationFunctionType.Exp
    DR = mybir.MatmulPerfMode.DoubleRow
    X = mybir.AxisListType.X
    MAX = mybir.AluOpType.max
    if True:
        ps3 = ps_3.tile([128, T], F32, tag="ps3")
        # Interleaved issue so the PE never waits on a tanh it just enabled:
        # mm1(g) ... then mm2(g-1) (whose t1 ran during mm1(g)), then
        # mm3(g-2) (whose t2 ran during mm1(g-1)/mm2(g-2)).
        t1s = {}
        t2s = {}
        for g in range(6):
            if g < 4:
                ps1 = ps_1.tile([128, T], F32, tag="ps1")
                for q in range(4):       # quad of 4 batches
                    q4 = 4 * g + q
                    wofs = 64 * (16 * tt + q4)
                    nc.tensor.matmul(
                        ps1[32 * q:32 * q + 32, :],
                        lhsT=weff[:, wofs:wofs + 32],
                        rhs=kt4[:, 200 * q4:200 * q4 + 200],
                        start=True, stop=False,
                        tile_position=(0, 32 * q), skip_group_check=True)
                    nc.tensor.matmul(
                        ps1[32 * q:32 * q + 32, :],
                        lhsT=weff[:, wofs + 32:wofs + 64],
                        rhs=kt4[:, 3200 + 200 * q4:3200 + 200 * q4 + 200],
                        start=False, stop=True,
                        tile_position=(0, 32 * q), skip_group_check=True)
                t1 = t1_pool.tile([128, T], BF16, tag="t1")
                G = 4 * tt + g
                nc.scalar.activation(t1[:], ps1[:], Tanh,
                                     bias=cbias[:, G:G + 1], scale=0.5)
                t1s[g] = t1
            if 1 <= g <= 4:
                ps2 = ps_2.tile([128, T], F32, tag="ps2")
                nc.tensor.matmul(ps2[:], lhsT=bd2[:], rhs=t1s[g - 1][:],
                                 start=True, stop=True)
                t2 = t2_pool.tile([128, T], BF16, tag="t2")
                nc.scalar.activation(t2[:], ps2[:], Tanh, bias=b2v[:],
                                     scale=0.5)
                t2s[g - 1] = t2
            if g >= 2:
                gg = g - 2
                nc.tensor.matmul(ps3[32 * gg:32 * gg + 32, :], lhsT=bd3[:],
                                 rhs=t2s[gg][:], start=True, stop=True,
                                 tile_position=(0, 32 * gg))

        # ---- softmax over free dim (cols parity-permuted; masks match) ----
        sc = sm_pool.tile([128, T], F32, tag="sc")
        nc.vector.tensor_copy(sc[:], negC[:])
        nc.vector.copy_predicated(sc[:], mask_t[:, T * tt:T * tt + T], ps3[:])
        e = sm_pool.tile([128, T], BF16, tag="e")
        nc.scalar.activation(e[:], sc[:], Exp,
                             accum_out=zstg[:, tt:tt + 1])
        return (tt, e)


def _attn_transp(nc, attn, id128, ps_a, at_pool):
    # ---- transpose exp-weights -> columns (bf16) ----
    psa = ps_a.tile([100, 256], BF16, tag="psa")
    nc.tensor.transpose(psa[0:100, 0:128], attn[:, 0:100], id128[:])
    nc.tensor.transpose(psa[0:100, 128:256], attn[:, 100:200], id128[:])
    at = at_pool.tile([100, 256], BF16, tag="at")
    nc.vector.tensor_copy(at[:], psa[:])
    return at


def _pool_out(nc, tt, sup, at, ps_p, stg_pool, out_d):
    if True:
        # ---- pooling: per super 2 matmuls (even/odd t), free=512 ----
        sup_r = sup[:].rearrange("p (s bb r d) -> p s r bb d", s=8, bb=8, r=2)
        stg = stg_pool.tile([128, 1024], BF16, tag="stg")
        # lhsT widened to 32 at-columns so every psum row is written
        # (extra rows are garbage the host ignores) -- no memset needed.
        # q==3 uses a window shifted -24 so it stays inside the 128-col
        # parity block; its valid rows sit at +24.
        for bank in range(2):
            psp = ps_p.tile([128, 512], F32, tag="psp")
            for q in range(4):
                s8 = 4 * bank + q
                c0 = 32 * (s8 // 2) + 8 * (s8 % 2)
                off = 24 if q == 3 else 0
                w = c0 - off
                nc.tensor.matmul(
                    psp[32 * q:32 * q + 32, :],
                    lhsT=at[0:100, w:w + 32],
                    rhs=sup_r[:, s8, 0],
                    start=True, stop=False, tile_position=(0, 32 * q),
                    skip_group_check=True)
                nc.tensor.matmul(
                    psp[32 * q:32 * q + 32, :],
                    lhsT=at[0:100, 128 + w:128 + w + 32],
                    rhs=sup_r[:, s8, 1],
                    start=False, stop=True, tile_position=(0, 32 * q),
                    skip_group_check=True)
            if bank == 0:
                nc.vector.tensor_copy(stg[:, 0:512], psp[:])
            else:
                nc.scalar.activation(stg[:, 512:1024], psp[:],
                                     mybir.ActivationFunctionType.Copy)
        nc.scalar.dma_start(out_d[tt], stg[:])


_NC_CACHE = {}


def _get_nc(ntt=NTT):
    if ntt not in _NC_CACHE:
        _NC_CACHE[ntt] = _build_kernel(ntt)
    return _NC_CACHE[ntt]


# parity permutation of t: col j<100 <-> t=2j, col 100+j <-> t=2j+1
_TPERM = np.concatenate([np.arange(0, T, 2), np.arange(1, T, 2)])


def make_core_inputs(queries, keys, key_masks, W1, b1, W2, b2, W3, b3,
                     core, ntt=NTT):
    """Host-side prep of one core's input map (all numpy)."""
    nb = ntt * TT_BATCHES
    cs = core * BC
    q = np.asarray(queries[cs:cs + nb, 0, :], dtype=np.float32)      # [nb,64]
    kf = np.asarray(keys[cs:cs + nb], dtype=np.float32)              # [nb,200,64]
    kbf = kf.astype(ml_dtypes.bfloat16)
    m = np.asarray(key_masks[cs:cs + nb, 0, :])                      # [nb,200]
    W1 = np.asarray(W1, np.float32); W2 = np.asarray(W2, np.float32)
    W3 = np.asarray(W3, np.float32)
    b1 = np.asarray(b1, np.float32); b2 = np.asarray(b2, np.float32)

    # ---- keys, t-major bf16 supers: [ntt, 100, 8192] ----
    # col 1024*s8 + 128*bb + 64*r + d <- keys[64*tt + 8*s8 + bb, 2*p + r, d]
    kk = kbf.reshape(ntt, 8, 8, 100, 2, D)          # [tt,s8,bb,p,r,d]
    ksup = np.ascontiguousarray(kk.transpose(0, 3, 1, 2, 4, 5)
                                ).reshape(ntt, 100, 8192)

    # ---- keys, feature-major fp8 quads: [ntt, 128, 6400] ----
    # part 64*h + d, col 400*q4 + 200*i + tp <- keys[4*q4 + 2*i + h, t, d]
    kp8 = kf[:, _TPERM, :].astype(NPFP8)            # [nb, 200(tp), 64]
    kq = kp8.reshape(ntt, 16, 2, 2, T, D)           # [tt,q4,i,h,tp,d]
    # slot-major halves: col = 3200*i + 200*q4 + tp (DoubleRow k-tile
    # stride must be a multiple of 16 bytes)
    kt4 = np.ascontiguousarray(kq.transpose(0, 3, 5, 2, 1, 4)
                               ).reshape(ntt, 128, 6400)

    # ---- per-batch effective W1 (DoubleRow layout, fp8) ----
    W1q, W1k, W1m, W1p = W1[0:64], W1[64:128], W1[128:192], W1[192:256]
    Weff = (W1k - W1m)[None] + q[:, :, None] * W1p[None]             # [nb,64,8]
    c = q @ (W1q + W1m) + b1                                         # [nb,8]
    # weff per quad: two [128,32] lhsT blocks (AB then CD), each block-
    # diag (A/C rows 0:64 -> cols +0:8|16:24, B/D rows 64:128 -> +8:16|24:32)
    weff = np.zeros((128, nb * 16), np.float32)
    wr = weff.reshape(128, nb // 4, 2, 32)          # [p, quad, half, 32]
    Wq = Weff.reshape(nb // 4, 2, 2, 64, 8)         # [quad, i, h, d, j]
    wr[0:64, :, 0, 0:8] = Wq[:, 0, 0].transpose(1, 0, 2)      # A
    wr[64:128, :, 0, 8:16] = Wq[:, 0, 1].transpose(1, 0, 2)   # B
    wr[0:64, :, 1, 16:24] = Wq[:, 1, 0].transpose(1, 0, 2)    # C
    wr[64:128, :, 1, 24:32] = Wq[:, 1, 1].transpose(1, 0, 2)  # D

    # ---- cbias [128, ntt*4]: col (4*tt+g), row 32*q + 8*l + j = 0.5*c ----
    cb = np.zeros((4, 4, 8, nb // 16), np.float32)  # [q, l, j, G]
    ci = 0.5 * c.reshape(nb // 16, 4, 4, 8)         # [G, q, l, j]
    cb[:, :, :, :] = ci.transpose(1, 2, 3, 0)
    cbias = np.ascontiguousarray(cb.reshape(128, nb // 16))

    # ---- bd2 [128,128]: [32q+8l+j, 32q+4l+cc] = 0.5*W2[j,cc] ----
    bd2 = np.zeros((128, 128), np.float32)
    for qq in range(4):
        for ll in range(4):
            r0 = 32 * qq + 8 * ll
            c0 = 32 * qq + 4 * ll
            bd2[r0:r0 + 8, c0:c0 + 4] = 0.5 * W2
    # ---- b2v [128,1]: row 32q+4l+cc = 0.5*(b2[cc] + 0.5*sum_j W2[j,cc]) ----
    b2f = 0.5 * (b2 + 0.5 * W2.sum(axis=0))
    b2v = np.zeros((128, 1), np.float32)
    for qq in range(4):
        for ll in range(4):
            r0 = 32 * qq + 4 * ll
            b2v[r0:r0 + 4, 0] = b2f
    # ---- bd3 [128,32]: [32q+4l+cc, 4q+l] = 0.5*W3[cc,0]; cols 16:32 zero
    bd3 = np.zeros((128, 32), np.float32)
    for qq in range(4):
        for ll in range(4):
            r0 = 32 * qq + 4 * ll
            bd3[r0:r0 + 4, 4 * qq + ll] = 0.5 * W3[:, 0]

    # ---- masks [128, ntt*T] u8, parity-permuted; row 32g+w = batch 16g+w --
    mperm = m[:, _TPERM].astype(np.uint8)           # [nb, T]
    mk = np.zeros((ntt, 4, 32, T), np.uint8)
    mk[:, :, 0:16, :] = mperm.reshape(ntt, 4, 16, T)
    masks = np.ascontiguousarray(
        mk.transpose(1, 2, 0, 3)).reshape(128, ntt * T)

    id128 = np.eye(128, dtype=np.float32)
    return {
        "ksup": ksup,
        "kt4": kt4,
        "weff": weff.astype(NPFP8),
        "cbias": cbias,
        "bd2": bd2.astype(ml_dtypes.bfloat16),
        "bd3": bd3.astype(ml_dtypes.bfloat16),
        "b2v": b2v,
        "masks": masks,
        "id128": id128.astype(ml_dtypes.bfloat16),
    }


def unpack_out(res, zsum, ntt=NTT):
    """res [ntt,128,1024](bf16, unnormalized): batch 64tt+32bank+8q+j is at
    [tt, 32q+off+j, 512*bank+64j : +64] (off=24 for q==3 else 0);
    zsum [128, ntt] f32: denominator at [32g+w, tt], batch 64tt+16g+w."""
    rr = np.asarray(res, np.float32).reshape(ntt, 4, 32, 1024)
    r = np.stack([rr[:, q, (24 if q == 3 else 0):(24 if q == 3 else 0) + 8]
                  for q in range(4)], axis=1).reshape(ntt, 4, 8, 2, 8, D)
    # axes: [tt, q, j, bank, jj(colblock), d]; valid jj == j
    d = np.diagonal(r, axis1=2, axis2=4)            # [ntt, q, bank, d, j]
    out = np.ascontiguousarray(
        d.transpose(0, 2, 1, 4, 3)).reshape(ntt * 64, D)
    z = np.asarray(zsum, np.float32).reshape(4, 32, ntt)[:, 0:16, :]
    z = np.ascontiguousarray(z.transpose(2, 0, 1)).reshape(ntt * 64)
    return out / z[:, None]


def kernel(queries, keys, key_masks, W1, b1, W2, b2, W3, b3):
    nc = _get_nc(NTT)
    in_maps = [make_core_inputs(queries, keys, key_masks, W1, b1, W2, b2,
                                W3, b3, core) for core in range(NCORES)]
    res = run_bass_kernel_spmd(nc, in_maps, list(range(NCORES)))
    outs = [unpack_out(res.results[c]["out"], res.results[c]["zsum"])
            for c in range(NCORES)]
    return np.concatenate(outs, axis=0).reshape(B, 1, D).astype(np.float32)
